# revision 11
# baseline (speedup 1.0000x reference)
"""Overlapping-windows kernel (tf.nn.conv1d with identity filter) for TRN2.

Full input x: [64, 2000, 26] f32. Full output: [64, 2000, 494] f32 where
out[b, t, w*26 + c] = x_pad[b, t + w, c]  (x zero-padded by 9 frames each side).

Sharding: data parallel over batch with HALO-OVERLAP — 8 examples per core,
and the per-core input is pre-tiled ON THE HOST into [128, 3718]: partition
p = e*16 + k holds zero-padded rows [k*125-9, k*125+134) of example e
(125 output rows + 9-row halos, flattened). Host cost: ~2 MB strided copy
per core. This makes the device side completely uniform: no edge-case DMAs,
no memsets, every load/store is a whole-grid 128-descriptor transfer.

Device pipeline (xt [128, 3718] -> y [8, 2000, 494]; the per-partition row
stride in y is RWC = 125*494 for ALL partitions since YROW == 16*RWC):

  Loads: 4 column-slice DMAs (128 big descriptors each) issued up front on
  the two HWDGE rings, small first slice so chunk 0 expands ASAP; each
  expand chunk gates only on the slice completing its columns.

  Expand: DVE tensor_copy per chunk (fused 4-dim AP, f32 2x_2P mode, ~2
  elem/cycle): ob[p, i*494 + w*26 + c] = tile[p, (start+i+w)*26 + c].
  Even row counts keep the 2x perf mode (odd drops to 1x — the trailing
  11-row chunks are odd; their extra cost hides under the drain).

  Store: ONE 128-partition DMA per chunk (never split by partition halves:
  partitions 0-63 map to only half the SDMA ports — half-rate tail).
  ~20 KB descriptors drain at ~425-430 GB/s aggregate (SBUF-AXI fabric
  rate). 13 chunks over 8 rotating buffers; WAR reuse via per-buffer
  semaphores; ring assignment roughly byte-balanced.

  Measured decomposition: ~7 us fixed NEFF preamble, drain 8.7 -> ~88.7 us
  at fabric rate (33.5 MB), ~1.9 us teardown => ~90.6 us.
"""

from contextlib import ExitStack

import numpy as np

import concourse.bass as bass
import concourse.mybir as mybir
from concourse.bass_utils import run_bass_kernel_spmd

# Problem constants (hardcoded per contract)
B_FULL = 64
T = 2000
C = 26
NCTX = 9
W = 2 * NCTX + 1          # 19
WC = W * C                # 494
N_CORES = 8
BL = B_FULL // N_CORES    # 8 examples per core
K = 16                    # row-chunks per example -> BL*K = 128 partitions
R = T // K                # 125 output rows per partition
RC = R * C                # 3250
FL = (R + 2 * NCTX) * C   # 3718 floats per partition (125+18 rows * 26)
RWC = R * WC              # 61750 floats per partition-row in y
F32 = mybir.dt.float32

SIZES = [4, 8, 10, 10, 10, 10, 10, 10, 10, 10, 11, 11, 11]   # 125 rows
STARTS = [sum(SIZES[:i]) for i in range(len(SIZES))]
# ring 0 = sync, 1 = scalar; byte-balanced including the load slices.
RINGS = [1, 1, 0, 1, 0, 1, 0, 1, 0, 1, 0, 1, 0]
NBUF = 8
OUTW = max(SIZES) * WC

# Column-slice loads; chunk c gates on the slice completing its columns.
SLICES = [(0, 572), (572, 1976), (1976, 2886), (2886, 3718)]
SLICE_RING = [0, 1, 0, 0]      # LA,LC1,LC2 on sync; LB on scalar
CHUNK_GATE = {0: 0, 1: 1, 6: 2, 10: 3}   # chunk -> slice index


def _build():
    nchunk = len(SIZES)
    nc = bass.Bass()
    xt = nc.dram_tensor("xt", [128, FL], F32, kind="ExternalInput")
    y = nc.dram_tensor("y", [BL, T, WC], F32, kind="ExternalOutput")

    with ExitStack() as ctx:
        tile = ctx.enter_context(nc.sbuf_tensor("tile", [128, FL], F32))
        obufs = [ctx.enter_context(
                     nc.sbuf_tensor(f"obuf{i}", [128, OUTW], F32))
                 for i in range(NBUF)]
        lsems = [ctx.enter_context(nc.semaphore(f"lsem{i}"))
                 for i in range(len(SLICES))]
        esem = ctx.enter_context(nc.semaphore("esem"))
        osems = [ctx.enter_context(nc.semaphore(f"osem{i}"))
                 for i in range(NBUF)]
        block = ctx.enter_context(nc.Block())
        th = tile[:].tensor
        xtt = xt[:].tensor
        yt = y[:].tensor

        def bulk_load(eng, s):
            c0, c1 = SLICES[s]
            n = c1 - c0
            src = bass.AP(tensor=xtt, offset=c0, ap=[[FL, 128], [1, n]])
            dst = bass.AP(tensor=th, offset=c0, ap=[[FL, 128], [1, n]])
            eng.dma_start(out=dst, in_=src).then_inc(lsems[s], 16)

        def out_dma(eng, c):
            start, cn = STARTS[c], SIZES[c]
            ob = obufs[c % NBUF][:].tensor
            src = bass.AP(tensor=ob, offset=0, ap=[[OUTW, 128], [1, cn * WC]])
            dst = bass.AP(tensor=yt, offset=start * WC,
                          ap=[[RWC, 128], [1, cn * WC]])
            eng.dma_start(out=dst, in_=src).then_inc(osems[c % NBUF], 16)

        @block.vector
        def _(vector):
            nuse = [0] * NBUF
            for c in range(nchunk):
                if c in CHUNK_GATE:
                    vector.wait_ge(lsems[CHUNK_GATE[c]], 16)
                b = c % NBUF
                if c >= NBUF:
                    # WAR: all prior out-DMAs of this buffer completed.
                    vector.wait_ge(osems[b], 16 * nuse[b])
                nuse[b] += 1
                start, cn = STARTS[c], SIZES[c]
                ob = obufs[b][:].tensor
                # ob[p, i*494 + w*26 + cc] = tile[p, (start + i + w)*26 + cc]
                src = bass.AP(tensor=th, offset=start * C,
                              ap=[[FL, 128], [C, cn], [C, W], [1, C]])
                dst = bass.AP(tensor=ob, offset=0,
                              ap=[[OUTW, 128], [WC, cn], [C, W], [1, C]])
                vector.tensor_copy(out=dst, in_=src).then_inc(esem, 1)

        @block.sync
        def _(sync):
            for s in range(len(SLICES)):
                if SLICE_RING[s] == 0:
                    bulk_load(sync, s)
            for c in range(nchunk):
                if RINGS[c] == 0:
                    sync.wait_ge(esem, c + 1)
                    out_dma(sync, c)
            # All stores (both rings) complete before the NEFF retires.
            for b in range(NBUF):
                ntot = len([c for c in range(nchunk) if c % NBUF == b])
                sync.wait_ge(osems[b], 16 * ntot)

        @block.scalar
        def _(scalar):
            for s in range(len(SLICES)):
                if SLICE_RING[s] == 1:
                    bulk_load(scalar, s)
            for c in range(nchunk):
                if RINGS[c] == 1:
                    scalar.wait_ge(esem, c + 1)
                    out_dma(scalar, c)

    return nc


_NC = None


def _get_nc():
    global _NC
    if _NC is None:
        _NC = _build()
    return _NC


def _host_tile(x_core: np.ndarray) -> np.ndarray:
    """[8, 2000, 26] -> [128, 3718]: halo-overlapped, zero-padded row tiles."""
    xp = np.pad(x_core, ((0, 0), (NCTX, NCTX), (0, 0)))
    xpf = np.ascontiguousarray(xp).reshape(BL, -1)   # [8, 52468]
    st = xpf.strides
    tl = np.lib.stride_tricks.as_strided(
        xpf, shape=(BL, K, FL), strides=(st[0], RC * 4, 4))
    return np.ascontiguousarray(tl.reshape(128, FL))


def run(x: np.ndarray, trace: bool = False):
    """Run the kernel on all 8 cores; returns (y_full, BassKernelResults)."""
    x = np.ascontiguousarray(x, dtype=np.float32)
    assert x.shape == (B_FULL, T, C), x.shape
    nc = _get_nc()
    in_maps = [
        {"xt": _host_tile(x[i * BL:(i + 1) * BL])} for i in range(N_CORES)
    ]
    res = run_bass_kernel_spmd(
        nc, in_maps, core_ids=list(range(N_CORES)), trace=trace
    )
    y = np.concatenate([res.results[i]["y"] for i in range(N_CORES)], axis=0)
    return y, res


def kernel(x: np.ndarray) -> np.ndarray:
    y, _ = run(x)
    return y


# revision 13
# speedup vs baseline: 1.0044x; 1.0044x over previous
"""Overlapping-windows kernel (tf.nn.conv1d with identity filter) for TRN2.

Full input x: [64, 2000, 26] f32. Full output: [64, 2000, 494] f32 where
out[b, t, w*26 + c] = x_pad[b, t + w, c]  (x zero-padded by 9 frames each side).

Sharding: data parallel over batch with HALO-OVERLAP — 8 examples per core,
and the per-core input is pre-tiled ON THE HOST into [128, 3718]: partition
p = e*16 + k holds zero-padded rows [k*125-9, k*125+134) of example e
(125 output rows + 9-row halos, flattened). Host cost: ~2 MB strided copy
per core. This makes the device side completely uniform: no edge-case DMAs,
no memsets, every load/store is a whole-grid 128-descriptor transfer.

Device pipeline (xt [128, 3718] -> y [8, 2000, 494]; the per-partition row
stride in y is RWC = 125*494 for ALL partitions since YROW == 16*RWC):

  Loads: 4 column-slice DMAs (128 big descriptors each) issued up front on
  the two HWDGE rings, small first slice so chunk 0 expands ASAP; each
  expand chunk gates only on the slice completing its columns.

  Expand: DVE tensor_copy per chunk (fused 4-dim AP, f32 2x_2P mode, ~2
  elem/cycle): ob[p, i*494 + w*26 + c] = tile[p, (start+i+w)*26 + c].
  Even row counts keep the 2x perf mode (odd drops to 1x — the trailing
  11-row chunks are odd; their extra cost hides under the drain).

  Store: ONE 128-partition DMA per chunk (never split by partition halves:
  partitions 0-63 map to only half the SDMA ports — half-rate tail).
  ~20 KB descriptors drain at ~425-430 GB/s aggregate (SBUF-AXI fabric
  rate). 13 chunks over 8 rotating buffers; WAR reuse via per-buffer
  semaphores; ring assignment roughly byte-balanced.

  Measured decomposition: ~7 us fixed NEFF preamble, drain 8.7 -> ~88.7 us
  at fabric rate (33.5 MB), ~1.9 us teardown => ~90.6 us.
"""

from contextlib import ExitStack

import numpy as np

import concourse.bass as bass
import concourse.mybir as mybir
from concourse.bass_utils import run_bass_kernel_spmd

# Problem constants (hardcoded per contract)
B_FULL = 64
T = 2000
C = 26
NCTX = 9
W = 2 * NCTX + 1          # 19
WC = W * C                # 494
N_CORES = 8
BL = B_FULL // N_CORES    # 8 examples per core
K = 16                    # row-chunks per example -> BL*K = 128 partitions
R = T // K                # 125 output rows per partition
RC = R * C                # 3250
FL = (R + 2 * NCTX) * C   # 3718 floats per partition (125+18 rows * 26)
RWC = R * WC              # 61750 floats per partition-row in y
F32 = mybir.dt.float32

SIZES = [4, 8, 10, 10, 10, 10, 10, 10, 10, 10, 11, 11, 11]   # 125 rows
STARTS = [sum(SIZES[:i]) for i in range(len(SIZES))]
# ring 0 = sync, 1 = scalar; byte-balanced including the load slices.
RINGS = [1, 1, 0, 1, 0, 1, 0, 1, 0, 1, 0, 1, 0]
NBUF = 8
OUTW = max(SIZES) * WC

# Column-slice loads; chunk c gates on the slice completing its columns.
SLICES = [(0, 572), (572, 1976), (1976, 2886), (2886, 3718)]
SLICE_RING = [0, 1, 0, 0]      # LA,LC1,LC2 on sync; LB on scalar
CHUNK_GATE = {0: 0, 1: 1, 6: 2, 10: 3}   # chunk -> slice index


def _build():
    nchunk = len(SIZES)
    # Trim framework pre/postamble: no partition-id register load (a ~1 us
    # TENSOR_LOAD on every engine), no monotonic-sem setup, no gpsimd
    # dge_drain at block exit (gpsimd issues no DMAs here).
    nc = bass.Bass(enable_partition_id=False, monotonic_sem_count=0)
    xt = nc.dram_tensor("xt", [128, FL], F32, kind="ExternalInput")
    y = nc.dram_tensor("y", [BL, T, WC], F32, kind="ExternalOutput")

    with ExitStack() as ctx:
        tile = ctx.enter_context(nc.sbuf_tensor("tile", [128, FL], F32))
        obufs = [ctx.enter_context(
                     nc.sbuf_tensor(f"obuf{i}", [128, OUTW], F32))
                 for i in range(NBUF)]
        lsems = [ctx.enter_context(nc.semaphore(f"lsem{i}"))
                 for i in range(len(SLICES))]
        esem = ctx.enter_context(nc.semaphore("esem"))
        osems = [ctx.enter_context(nc.semaphore(f"osem{i}"))
                 for i in range(NBUF)]
        block = ctx.enter_context(nc.Block(no_gpsimd_drain=True))
        th = tile[:].tensor
        xtt = xt[:].tensor
        yt = y[:].tensor

        def bulk_load(eng, s):
            c0, c1 = SLICES[s]
            n = c1 - c0
            src = bass.AP(tensor=xtt, offset=c0, ap=[[FL, 128], [1, n]])
            dst = bass.AP(tensor=th, offset=c0, ap=[[FL, 128], [1, n]])
            eng.dma_start(out=dst, in_=src).then_inc(lsems[s], 16)

        def out_dma(eng, c):
            start, cn = STARTS[c], SIZES[c]
            ob = obufs[c % NBUF][:].tensor
            src = bass.AP(tensor=ob, offset=0, ap=[[OUTW, 128], [1, cn * WC]])
            dst = bass.AP(tensor=yt, offset=start * WC,
                          ap=[[RWC, 128], [1, cn * WC]])
            eng.dma_start(out=dst, in_=src).then_inc(osems[c % NBUF], 16)

        @block.vector
        def _(vector):
            nuse = [0] * NBUF
            for c in range(nchunk):
                if c in CHUNK_GATE:
                    vector.wait_ge(lsems[CHUNK_GATE[c]], 16)
                b = c % NBUF
                if c >= NBUF:
                    # WAR: all prior out-DMAs of this buffer completed.
                    vector.wait_ge(osems[b], 16 * nuse[b])
                nuse[b] += 1
                start, cn = STARTS[c], SIZES[c]
                ob = obufs[b][:].tensor
                # ob[p, i*494 + w*26 + cc] = tile[p, (start + i + w)*26 + cc]
                src = bass.AP(tensor=th, offset=start * C,
                              ap=[[FL, 128], [C, cn], [C, W], [1, C]])
                dst = bass.AP(tensor=ob, offset=0,
                              ap=[[OUTW, 128], [WC, cn], [C, W], [1, C]])
                vector.tensor_copy(out=dst, in_=src).then_inc(esem, 1)

        @block.sync
        def _(sync):
            for s in range(len(SLICES)):
                if SLICE_RING[s] == 0:
                    bulk_load(sync, s)
            for c in range(nchunk):
                if RINGS[c] == 0:
                    sync.wait_ge(esem, c + 1)
                    out_dma(sync, c)
            # All stores (both rings) complete before the NEFF retires.
            for b in range(NBUF):
                ntot = len([c for c in range(nchunk) if c % NBUF == b])
                sync.wait_ge(osems[b], 16 * ntot)

        @block.scalar
        def _(scalar):
            for s in range(len(SLICES)):
                if SLICE_RING[s] == 1:
                    bulk_load(scalar, s)
            for c in range(nchunk):
                if RINGS[c] == 1:
                    scalar.wait_ge(esem, c + 1)
                    out_dma(scalar, c)

    return nc


_NC = None


def _get_nc():
    global _NC
    if _NC is None:
        _NC = _build()
    return _NC


def _host_tile(x_core: np.ndarray) -> np.ndarray:
    """[8, 2000, 26] -> [128, 3718]: halo-overlapped, zero-padded row tiles."""
    xp = np.pad(x_core, ((0, 0), (NCTX, NCTX), (0, 0)))
    xpf = np.ascontiguousarray(xp).reshape(BL, -1)   # [8, 52468]
    st = xpf.strides
    tl = np.lib.stride_tricks.as_strided(
        xpf, shape=(BL, K, FL), strides=(st[0], RC * 4, 4))
    return np.ascontiguousarray(tl.reshape(128, FL))


def run(x: np.ndarray, trace: bool = False):
    """Run the kernel on all 8 cores; returns (y_full, BassKernelResults)."""
    x = np.ascontiguousarray(x, dtype=np.float32)
    assert x.shape == (B_FULL, T, C), x.shape
    nc = _get_nc()
    in_maps = [
        {"xt": _host_tile(x[i * BL:(i + 1) * BL])} for i in range(N_CORES)
    ]
    res = run_bass_kernel_spmd(
        nc, in_maps, core_ids=list(range(N_CORES)), trace=trace
    )
    y = np.concatenate([res.results[i]["y"] for i in range(N_CORES)], axis=0)
    return y, res


def kernel(x: np.ndarray) -> np.ndarray:
    y, _ = run(x)
    return y
